# revision 8
# baseline (speedup 1.0000x reference)
"""Trainium2 Bass kernel for nn_CrossAttention (B=8, N=16384, D=128).

Math per batch b (reference):
    xt      = x1[b].T                      # [D, N]
    keys    = softmax(xt, axis=N)          # row softmax over N
    queries = softmax(xt, axis=D)          # col softmax over D
    values  = x2[b].T                      # [Dv, N]
    context = keys @ values.T              # [D, Dv]
    attended= context.T @ queries          # [Dv, N]
    eff     = (w @ attended).T + bias      # [N, 2D]
    out     = LayerNorm(eff) * gamma + beta

Kernel strategy (one batch per NeuronCore, 8 cores data-parallel):
  * No max-subtraction in softmax (randn inputs, exp is safe in fp32).
  * context matmul in natural layout: ctx[d,v] = sum_n E1[n,d] V[n,v]
    with a ones-column on V giving s1[d] = sum_n E1[n,d] for free;
    normalize by 1/s1 as a per-partition scalar.
  * g' = context @ w^T + bias  (bias folding is exact because the
    queries softmax weights sum to 1 over d).
  * U[n,o] = sum_d e2[d,n] * g'[d,o],  e2 = E1^T via PE transpose.
    Two extra g' columns give sum_o U/256 and z[n] = sum_d e2[d,n].
  * LayerNorm folded into per-partition scalars with no reciprocal:
      q = rsqrt((S2/256 - m^2) + eps*z^2);  out = U*q - m*q
    where S2 = sum_o U^2 (one square-reduce pass), m = sum_o U/256.
"""

import sys

sys.path.insert(0, "/opt/trn_rl_repo")

import numpy as np

B = 8
N = 16384
D = 128
O = 256
NCHUNK = N // 128       # 128 chunks of 128 rows
SUPER = 8               # chunks per stage-A supertile
NSUPER = NCHUNK // SUPER
GRP = 4                 # chunks per stage-B psum group
NGRP = NCHUNK // GRP
TMBLK = 2               # groups per tiny-math batch
LN_EPS = 1e-5

_NC_CACHE = {}


def build_bass(n_iters: int = 1):
    import concourse.bacc as bacc
    import concourse.bass as bass
    import concourse.tile as tile
    from concourse import mybir
    from contextlib import ExitStack

    f32 = mybir.dt.float32
    AF = mybir.ActivationFunctionType
    ALU = mybir.AluOpType

    nc = bacc.Bacc("TRN2")
    x1e = nc.dram_tensor("x1", [N, D], f32, kind="ExternalInput")
    x2e = nc.dram_tensor("x2", [N, D], f32, kind="ExternalInput")
    wte = nc.dram_tensor("wt", [D, O], f32, kind="ExternalInput")
    bve = nc.dram_tensor("bv", [O], f32, kind="ExternalInput")
    oute = nc.dram_tensor("out", [N, O], f32, kind="ExternalOutput")
    ideye = nc.inline_tensor(np.eye(128, dtype=np.float32), name="ident_const")

    with tile.TileContext(nc) as tc:
        ctx = ExitStack()
        singles = ctx.enter_context(tc.tile_pool(name="singles", bufs=1))
        p_x1 = ctx.enter_context(tc.tile_pool(name="p_x1", bufs=3))
        p_v = ctx.enter_context(tc.tile_pool(name="p_v", bufs=3))
        p_e1 = ctx.enter_context(tc.tile_pool(name="p_e1", bufs=3))
        p_o = ctx.enter_context(tc.tile_pool(name="p_o", bufs=3))
        p_scr = ctx.enter_context(tc.tile_pool(name="p_scr", bufs=2))
        p_tiny = ctx.enter_context(tc.tile_pool(name="p_tiny", bufs=4))
        p_small = ctx.enter_context(tc.tile_pool(name="p_small", bufs=2))

        # one-time loads
        ident = singles.tile([128, 128], f32)
        nc.sync.dma_start(out=ident, in_=ideye[:, :])
        b_bcast = singles.tile([128, O], f32)
        bv_ap = bve[:]
        nc.sync.dma_start(
            out=b_bcast,
            in_=bass.AP(tensor=bv_ap.tensor, offset=bv_ap.offset,
                        ap=[[0, 128]] + list(bv_ap.ap)),
        )
        wts = singles.tile([128, O], f32)
        nc.sync.dma_start(out=wts, in_=wte[:, :])

        # persistent per-iteration state
        e2store = singles.tile([128, NCHUNK, 128], f32)    # [d, chunk, n_local]
        gaug = singles.tile([128, O + 2], f32)     # g' | rowavg(g') | ones
        ctxn = singles.tile([128, 128], f32)
        ctxT = singles.tile([128, 128], f32)
        stg_s2 = singles.tile([128, NCHUNK], f32)          # sum_o U^2
        stg_usz = singles.tile([128, NCHUNK, 2], f32)      # sum_o U/256 | z
        stg_q = singles.tile([128, NCHUNK], f32)           # rz*rs
        stg_nmq = singles.tile([128, NCHUNK], f32)         # -mean*rs

        def body(_iv=None):
            # ---------------- stage A + bridge (psum pools scoped) --------
            with tc.tile_pool(name="ps_ctx", bufs=1, space="PSUM") as ps_ctx, \
                 tc.tile_pool(name="ps_tr", bufs=2, space="PSUM") as ps_tr, \
                 tc.tile_pool(name="ps_g", bufs=1, space="PSUM") as ps_g:
                ctx_ps = ps_ctx.tile([128, D + 1], f32)
                for s in range(NSUPER):
                    r0 = s * SUPER * 128
                    r1 = r0 + SUPER * 128
                    x1s = p_x1.tile([128, SUPER, D], f32, tag="x1s")
                    nc.sync.dma_start(
                        out=x1s,
                        in_=x1e[r0:r1, :].rearrange("(c p) d -> p c d", p=128))
                    vaug = p_v.tile([128, SUPER, D + 1], f32, tag="vaug")
                    nc.sync.dma_start(
                        out=vaug[:, :, 0:D],
                        in_=x2e[r0:r1, :].rearrange("(c p) d -> p c d", p=128))
                    nc.gpsimd.memset(vaug[:, :, D:D + 1], 1.0)
                    e1s = p_e1.tile([128, SUPER, D], f32, tag="e1s")
                    nc.scalar.activation(out=e1s, in_=x1s, func=AF.Exp)
                    tr = None
                    for c8 in range(SUPER):
                        c = s * SUPER + c8
                        nc.tensor.matmul(
                            out=ctx_ps, lhsT=e1s[:, c8, :], rhs=vaug[:, c8, :],
                            start=(c == 0), stop=(c == NCHUNK - 1),
                            skip_group_check=True)
                        if c8 % 4 == 0:
                            tr = ps_tr.tile([128, 4, 128], f32, tag="tr")
                        nc.tensor.transpose(
                            out=tr[:, c8 % 4, :], in_=e1s[:, c8, :],
                            identity=ident)
                        if c8 % 4 == 3:
                            nc.vector.tensor_copy(
                                out=e2store[:, c - 3:c + 1, :], in_=tr)

                # ---- bridge: context -> g' ------------------------------
                rcp = p_small.tile([128, 1], f32, tag="rcp")
                nc.vector.reciprocal(out=rcp, in_=ctx_ps[:, D:D + 1])
                nc.vector.tensor_scalar_mul(
                    out=ctxn, in0=ctx_ps[:, 0:D], scalar1=rcp)
                trc = ps_tr.tile([128, 4, 128], f32, tag="tr")
                nc.tensor.transpose(out=trc[:, 0, :], in_=ctxn, identity=ident)
                nc.vector.tensor_copy(out=ctxT, in_=trc[:, 0, :])
                g_ps = ps_g.tile([128, O], f32)
                nc.tensor.matmul(out=g_ps, lhsT=ctxT, rhs=wts)
                nc.vector.tensor_add(out=gaug[:, 0:O], in0=g_ps, in1=b_bcast)
                scr0 = p_scr.tile([128, O], f32, tag="scr")
                nc.vector.tensor_tensor_reduce(
                    out=scr0, in0=gaug[:, 0:O], in1=gaug[:, 0:O],
                    scale=1.0 / O, scalar=0.0, op0=ALU.bypass, op1=ALU.add,
                    accum_out=gaug[:, O:O + 1])
                nc.gpsimd.memset(gaug[:, O + 1:O + 2], 1.0)

            # ---------------- stage B ------------------------------------
            with tc.tile_pool(name="ps_u", bufs=2, space="PSUM") as ps_u:
                u_tiles = {}
                for g in range(NGRP):
                    u_ps = ps_u.tile([128, GRP, 512], f32, tag="u")
                    u_tiles[g] = u_ps
                    for j in range(GRP):
                        c = g * GRP + j
                        nc.tensor.matmul(
                            out=u_ps[:, j, 0:O + 2], lhsT=e2store[:, c, :],
                            rhs=gaug)
                        # S2 = sum_o U^2 : alternate DVE / ACT per chunk
                        scr = p_scr.tile([128, O], f32, tag="scr")
                        if c % 2 == 0:
                            nc.vector.tensor_tensor_reduce(
                                out=scr, in0=u_ps[:, j, 0:O],
                                in1=u_ps[:, j, 0:O], scale=1.0, scalar=0.0,
                                op0=ALU.mult, op1=ALU.add,
                                accum_out=stg_s2[:, c:c + 1])
                        else:
                            nc.scalar.activation(
                                out=scr, in_=u_ps[:, j, 0:O], func=AF.Square,
                                accum_out=stg_s2[:, c:c + 1])
                    # extract [sum_o U/256 | z] columns for the group
                    nc.vector.tensor_copy(
                        out=stg_usz[:, g * GRP:(g + 1) * GRP, :],
                        in_=u_ps[:, :, O:O + 2])

                    if g % TMBLK != TMBLK - 1:
                        continue
                    if True:
                        # tiny-math for chunks [c0, c0+TMBLK*GRP)
                        c0 = (g - TMBLK + 1) * GRP
                        sl = slice(c0, c0 + TMBLK * GRP)
                        nb = TMBLK * GRP
                        mh = stg_usz[:, sl, 0]     # sum_o U / 256
                        zz = stg_usz[:, sl, 1]     # z
                        t1 = p_tiny.tile([128, nb], f32, tag="t1")
                        t2 = p_tiny.tile([128, nb], f32, tag="t2")
                        # t1 = S2/256 - mh^2 + eps*z^2   (= var * z^2)
                        nc.gpsimd.tensor_mul(out=t2, in0=mh, in1=mh)
                        nc.gpsimd.tensor_scalar(
                            out=t1, in0=stg_s2[:, sl], scalar1=1.0 / O,
                            scalar2=None, op0=ALU.mult)
                        nc.gpsimd.tensor_sub(out=t1, in0=t1, in1=t2)
                        nc.gpsimd.tensor_mul(out=t2, in0=zz, in1=zz)
                        nc.gpsimd.tensor_scalar(
                            out=t2, in0=t2, scalar1=LN_EPS, scalar2=None,
                            op0=ALU.mult)
                        nc.gpsimd.tensor_add(out=t1, in0=t1, in1=t2)
                        nc.scalar.activation(
                            out=t2, in_=t1, func=AF.Sqrt)
                        nc.vector.reciprocal(out=stg_q[:, sl], in_=t2)
                        # nmq = -mh * q
                        nc.gpsimd.tensor_mul(
                            out=stg_nmq[:, sl], in0=mh, in1=stg_q[:, sl])
                        nc.gpsimd.tensor_scalar(
                            out=stg_nmq[:, sl], in0=stg_nmq[:, sl],
                            scalar1=-1.0, scalar2=None, op0=ALU.mult)
                    # emit: out = U*q - mean*rs (engine alternates)
                    for gg in range(g - TMBLK + 1, g + 1):
                        ug = u_tiles.pop(gg)
                        osb = p_o.tile([128, GRP, O], f32, tag="osb")
                        for j in range(GRP):
                            c = gg * GRP + j
                            if c % 2 == 0:
                                nc.scalar.activation(
                                    out=osb[:, j, :], in_=ug[:, j, 0:O],
                                    func=AF.Identity, scale=stg_q[:, c:c + 1],
                                    bias=stg_nmq[:, c:c + 1])
                            else:
                                nc.vector.tensor_scalar(
                                    out=osb[:, j, :], in0=ug[:, j, 0:O],
                                    scalar1=stg_q[:, c:c + 1],
                                    scalar2=stg_nmq[:, c:c + 1],
                                    op0=ALU.mult, op1=ALU.add)
                        nc.sync.dma_start(
                            out=oute[gg * GRP * 128:(gg + 1) * GRP * 128, :]
                            .rearrange("(c p) o -> p c o", p=128),
                            in_=osb)

        if n_iters == 1:
            body()
        else:
            with tc.For_i(0, n_iters, 1) as iv:
                body(iv)
        ctx.close()

    nc.finalize()
    return nc


def _get_nc(n_iters=1):
    if n_iters not in _NC_CACHE:
        _NC_CACHE[n_iters] = build_bass(n_iters)
    return _NC_CACHE[n_iters]


def _host_inputs(x1, x2, w_reproj, b_reproj):
    x1 = np.ascontiguousarray(np.asarray(x1, dtype=np.float32))
    x2 = np.ascontiguousarray(np.asarray(x2, dtype=np.float32))
    w = np.asarray(w_reproj, dtype=np.float32).reshape(O, D)
    wt = np.ascontiguousarray(w.T)
    bv = np.ascontiguousarray(np.asarray(b_reproj, dtype=np.float32))
    return [
        {"x1": x1[b], "x2": x2[b], "wt": wt, "bv": bv}
        for b in range(B)
    ]


def kernel(x1, x2, w_reproj, b_reproj, ln_gamma, ln_beta):
    from concourse.bass_utils import run_bass_kernel_spmd

    nc = _get_nc(1)
    in_maps = _host_inputs(x1, x2, w_reproj, b_reproj)
    res = run_bass_kernel_spmd(nc, in_maps, list(range(B)))
    out = np.stack([res.results[b]["out"] for b in range(B)], axis=0)
    g = np.asarray(ln_gamma, dtype=np.float32)
    bb = np.asarray(ln_beta, dtype=np.float32)
    if not (np.all(g == 1.0) and np.all(bb == 0.0)):
        out = (out * g + bb).astype(np.float32)
    return out


# revision 9
# speedup vs baseline: 1.3630x; 1.3630x over previous
"""Trainium2 Bass kernel for nn_CrossAttention (B=8, N=16384, D=128).

Math per batch b (reference):
    xt      = x1[b].T                      # [D, N]
    keys    = softmax(xt, axis=N)          # row softmax over N
    queries = softmax(xt, axis=D)          # col softmax over D
    values  = x2[b].T                      # [Dv, N]
    context = keys @ values.T              # [D, Dv]
    attended= context.T @ queries          # [Dv, N]
    eff     = (w @ attended).T + bias      # [N, 2D]
    out     = LayerNorm(eff) * gamma + beta

Kernel strategy (one batch per NeuronCore, 8 cores data-parallel):
  * No max-subtraction in softmax (randn inputs, exp is safe in fp32).
  * context matmul in natural layout: ctx[d,v] = sum_n E1[n,d] V[n,v]
    with a ones-column on V giving s1[d] = sum_n E1[n,d] for free;
    normalize by 1/s1 as a per-partition scalar.
  * g' = context @ w^T + bias  (bias folding is exact because the
    queries softmax weights sum to 1 over d).
  * U[n,o] = sum_d e2[d,n] * g'[d,o],  e2 = E1^T via PE transpose.
    Two extra g' columns give sum_o U/256 and z[n] = sum_d e2[d,n].
  * LayerNorm folded into per-partition scalars with no reciprocal:
      q = rsqrt((S2/256 - m^2) + eps*z^2);  out = U*q - m*q
    where S2 = sum_o U^2 (one square-reduce pass), m = sum_o U/256.
"""

import sys

sys.path.insert(0, "/opt/trn_rl_repo")

import numpy as np

B = 8
N = 16384
D = 128
O = 256
NCHUNK = N // 128       # 128 chunks of 128 rows
SUPER = 8               # chunks per stage-A supertile
NSUPER = NCHUNK // SUPER
GRP = 4                 # chunks per stage-B psum group
NGRP = NCHUNK // GRP
TMBLK = 1               # groups per tiny-math batch
LN_EPS = 1e-5

_NC_CACHE = {}


def build_bass(n_iters: int = 1):
    import concourse.bacc as bacc
    import concourse.bass as bass
    import concourse.tile as tile
    from concourse import mybir
    from contextlib import ExitStack

    f32 = mybir.dt.float32
    AF = mybir.ActivationFunctionType
    ALU = mybir.AluOpType

    nc = bacc.Bacc("TRN2")
    x1e = nc.dram_tensor("x1", [N, D], f32, kind="ExternalInput")
    x2e = nc.dram_tensor("x2", [N, D], f32, kind="ExternalInput")
    wte = nc.dram_tensor("wt", [D, O], f32, kind="ExternalInput")
    bve = nc.dram_tensor("bv", [O], f32, kind="ExternalInput")
    oute = nc.dram_tensor("out", [N, O], f32, kind="ExternalOutput")
    ideye = nc.inline_tensor(np.eye(128, dtype=np.float32), name="ident_const")

    with tile.TileContext(nc) as tc:
        ctx = ExitStack()
        singles = ctx.enter_context(tc.tile_pool(name="singles", bufs=1))
        p_x1 = ctx.enter_context(tc.tile_pool(name="p_x1", bufs=3))
        p_v = ctx.enter_context(tc.tile_pool(name="p_v", bufs=3))
        p_e1 = ctx.enter_context(tc.tile_pool(name="p_e1", bufs=3))
        p_o = ctx.enter_context(tc.tile_pool(name="p_o", bufs=3))
        p_scr = ctx.enter_context(tc.tile_pool(name="p_scr", bufs=2))
        p_tiny = ctx.enter_context(tc.tile_pool(name="p_tiny", bufs=4))
        p_small = ctx.enter_context(tc.tile_pool(name="p_small", bufs=2))

        # one-time loads
        ident = singles.tile([128, 128], f32)
        nc.sync.dma_start(out=ident, in_=ideye[:, :])
        b_bcast = singles.tile([128, O], f32)
        bv_ap = bve[:]
        nc.sync.dma_start(
            out=b_bcast,
            in_=bass.AP(tensor=bv_ap.tensor, offset=bv_ap.offset,
                        ap=[[0, 128]] + list(bv_ap.ap)),
        )
        wts = singles.tile([128, O], f32)
        nc.sync.dma_start(out=wts, in_=wte[:, :])

        # persistent per-iteration state
        e2store = singles.tile([128, NCHUNK, 128], f32)    # [d, chunk, n_local]
        gaug = singles.tile([128, O + 2], f32)     # g' | rowavg(g') | ones
        ctxn = singles.tile([128, 128], f32)
        ctxT = singles.tile([128, 128], f32)
        stg_s2 = singles.tile([128, NCHUNK], f32)          # sum_o U^2
        stg_usz = singles.tile([128, NCHUNK, 2], f32)      # sum_o U/256 | z
        stg_q = singles.tile([128, NCHUNK], f32)           # rz*rs
        stg_nmq = singles.tile([128, NCHUNK], f32)         # -mean*rs

        def body(_iv=None):
            # ---------------- stage A + bridge (psum pools scoped) --------
            with tc.tile_pool(name="ps_ctx", bufs=1, space="PSUM") as ps_ctx, \
                 tc.tile_pool(name="ps_tr", bufs=2, space="PSUM") as ps_tr, \
                 tc.tile_pool(name="ps_g", bufs=1, space="PSUM") as ps_g:
                ctx_ps = ps_ctx.tile([128, D + 1], f32)
                for s in range(NSUPER):
                    r0 = s * SUPER * 128
                    r1 = r0 + SUPER * 128
                    x1s = p_x1.tile([128, SUPER, D], f32, tag="x1s")
                    nc.sync.dma_start(
                        out=x1s,
                        in_=x1e[r0:r1, :].rearrange("(c p) d -> p c d", p=128))
                    vaug = p_v.tile([128, SUPER, D + 1], f32, tag="vaug")
                    nc.gpsimd.dma_start(
                        out=vaug[:, :, 0:D],
                        in_=x2e[r0:r1, :].rearrange("(c p) d -> p c d", p=128))
                    nc.gpsimd.memset(vaug[:, :, D:D + 1], 1.0)
                    e1s = p_e1.tile([128, SUPER, D], f32, tag="e1s")
                    nc.scalar.activation(out=e1s, in_=x1s, func=AF.Exp)
                    tr = None
                    for c8 in range(SUPER):
                        c = s * SUPER + c8
                        nc.tensor.matmul(
                            out=ctx_ps, lhsT=e1s[:, c8, :], rhs=vaug[:, c8, :],
                            start=(c == 0), stop=(c == NCHUNK - 1),
                            skip_group_check=True)
                        if c8 % 4 == 0:
                            tr = ps_tr.tile([128, 4, 128], f32, tag="tr")
                        nc.tensor.transpose(
                            out=tr[:, c8 % 4, :], in_=e1s[:, c8, :],
                            identity=ident)
                        if c8 % 4 == 3:
                            nc.vector.tensor_copy(
                                out=e2store[:, c - 3:c + 1, :], in_=tr)

                # ---- bridge: context -> g' ------------------------------
                rcp = p_small.tile([128, 1], f32, tag="rcp")
                nc.vector.reciprocal(out=rcp, in_=ctx_ps[:, D:D + 1])
                nc.vector.tensor_scalar_mul(
                    out=ctxn, in0=ctx_ps[:, 0:D], scalar1=rcp)
                trc = ps_tr.tile([128, 4, 128], f32, tag="tr")
                nc.tensor.transpose(out=trc[:, 0, :], in_=ctxn, identity=ident)
                nc.vector.tensor_copy(out=ctxT, in_=trc[:, 0, :])
                g_ps = ps_g.tile([128, O], f32)
                nc.tensor.matmul(out=g_ps, lhsT=ctxT, rhs=wts)
                nc.vector.tensor_add(out=gaug[:, 0:O], in0=g_ps, in1=b_bcast)
                scr0 = p_scr.tile([128, O], f32, tag="scr")
                nc.vector.tensor_tensor_reduce(
                    out=scr0, in0=gaug[:, 0:O], in1=gaug[:, 0:O],
                    scale=1.0 / O, scalar=0.0, op0=ALU.bypass, op1=ALU.add,
                    accum_out=gaug[:, O:O + 1])
                nc.gpsimd.memset(gaug[:, O + 1:O + 2], 1.0)

            # ---------------- stage B ------------------------------------
            with tc.tile_pool(name="ps_u", bufs=2, space="PSUM") as ps_u:
                u_tiles = {}
                for g in range(NGRP):
                    u_ps = ps_u.tile([128, GRP, 512], f32, tag="u")
                    u_tiles[g] = u_ps
                    for j in range(GRP):
                        c = g * GRP + j
                        nc.tensor.matmul(
                            out=u_ps[:, j, 0:O + 2], lhsT=e2store[:, c, :],
                            rhs=gaug)
                        # S2 = sum_o U^2 : alternate DVE / ACT per chunk
                        scr = p_scr.tile([128, O], f32, tag="scr")
                        if c % 2 == 0:
                            nc.vector.tensor_tensor_reduce(
                                out=scr, in0=u_ps[:, j, 0:O],
                                in1=u_ps[:, j, 0:O], scale=1.0, scalar=0.0,
                                op0=ALU.mult, op1=ALU.add,
                                accum_out=stg_s2[:, c:c + 1])
                        else:
                            nc.scalar.activation(
                                out=scr, in_=u_ps[:, j, 0:O], func=AF.Square,
                                accum_out=stg_s2[:, c:c + 1])
                    # extract [sum_o U/256 | z] columns for the group
                    nc.vector.tensor_copy(
                        out=stg_usz[:, g * GRP:(g + 1) * GRP, :],
                        in_=u_ps[:, :, O:O + 2])

                    if g % TMBLK != TMBLK - 1:
                        continue
                    if True:
                        # tiny-math for chunks [c0, c0+TMBLK*GRP)
                        c0 = (g - TMBLK + 1) * GRP
                        sl = slice(c0, c0 + TMBLK * GRP)
                        nb = TMBLK * GRP
                        mh = stg_usz[:, sl, 0]     # sum_o U / 256
                        zz = stg_usz[:, sl, 1]     # z
                        t1 = p_tiny.tile([128, nb], f32, tag="t1")
                        t2 = p_tiny.tile([128, nb], f32, tag="t2")
                        # t1 = S2/256 - mh^2 + eps*z^2   (= var * z^2)
                        nc.gpsimd.tensor_mul(out=t2, in0=mh, in1=mh)
                        nc.gpsimd.tensor_scalar(
                            out=t1, in0=stg_s2[:, sl], scalar1=1.0 / O,
                            scalar2=None, op0=ALU.mult)
                        nc.gpsimd.tensor_sub(out=t1, in0=t1, in1=t2)
                        nc.gpsimd.tensor_mul(out=t2, in0=zz, in1=zz)
                        nc.gpsimd.tensor_scalar(
                            out=t2, in0=t2, scalar1=LN_EPS, scalar2=None,
                            op0=ALU.mult)
                        nc.gpsimd.tensor_add(out=t1, in0=t1, in1=t2)
                        nc.scalar.activation(
                            out=t2, in_=t1, func=AF.Sqrt)
                        nc.vector.reciprocal(out=stg_q[:, sl], in_=t2)
                        # nmq = -mh * q
                        nc.gpsimd.tensor_mul(
                            out=stg_nmq[:, sl], in0=mh, in1=stg_q[:, sl])
                        nc.gpsimd.tensor_scalar(
                            out=stg_nmq[:, sl], in0=stg_nmq[:, sl],
                            scalar1=-1.0, scalar2=None, op0=ALU.mult)
                    # emit: out = U*q - mean*rs (engine alternates)
                    for gg in range(g - TMBLK + 1, g + 1):
                        ug = u_tiles.pop(gg)
                        osb = p_o.tile([128, GRP, O], f32, tag="osb")
                        for j in range(GRP):
                            c = gg * GRP + j
                            if c % 2 == 0:
                                nc.scalar.activation(
                                    out=osb[:, j, :], in_=ug[:, j, 0:O],
                                    func=AF.Identity, scale=stg_q[:, c:c + 1],
                                    bias=stg_nmq[:, c:c + 1])
                            else:
                                nc.vector.tensor_scalar(
                                    out=osb[:, j, :], in0=ug[:, j, 0:O],
                                    scalar1=stg_q[:, c:c + 1],
                                    scalar2=stg_nmq[:, c:c + 1],
                                    op0=ALU.mult, op1=ALU.add)
                        dma_eng = nc.sync if gg % 2 == 0 else nc.gpsimd
                        dma_eng.dma_start(
                            out=oute[gg * GRP * 128:(gg + 1) * GRP * 128, :]
                            .rearrange("(c p) o -> p c o", p=128),
                            in_=osb)

        if n_iters == 1:
            body()
        else:
            with tc.For_i(0, n_iters, 1) as iv:
                body(iv)
        ctx.close()

    nc.finalize()
    return nc


def _get_nc(n_iters=1):
    if n_iters not in _NC_CACHE:
        _NC_CACHE[n_iters] = build_bass(n_iters)
    return _NC_CACHE[n_iters]


def _host_inputs(x1, x2, w_reproj, b_reproj):
    x1 = np.ascontiguousarray(np.asarray(x1, dtype=np.float32))
    x2 = np.ascontiguousarray(np.asarray(x2, dtype=np.float32))
    w = np.asarray(w_reproj, dtype=np.float32).reshape(O, D)
    wt = np.ascontiguousarray(w.T)
    bv = np.ascontiguousarray(np.asarray(b_reproj, dtype=np.float32))
    return [
        {"x1": x1[b], "x2": x2[b], "wt": wt, "bv": bv}
        for b in range(B)
    ]


def kernel(x1, x2, w_reproj, b_reproj, ln_gamma, ln_beta):
    from concourse.bass_utils import run_bass_kernel_spmd

    nc = _get_nc(1)
    in_maps = _host_inputs(x1, x2, w_reproj, b_reproj)
    res = run_bass_kernel_spmd(nc, in_maps, list(range(B)))
    out = np.stack([res.results[b]["out"] for b in range(B)], axis=0)
    g = np.asarray(ln_gamma, dtype=np.float32)
    bb = np.asarray(ln_beta, dtype=np.float32)
    if not (np.all(g == 1.0) and np.all(bb == 0.0)):
        out = (out * g + bb).astype(np.float32)
    return out
